# revision 1
# baseline (speedup 1.0000x reference)
"""Trainium2 Bass kernel for the segmented-attention block.

Reference computation (per batch row b of x [B, S*D]):
    xs = x[b].reshape(S, D)
    q_s = xs[s] @ Q[s]; k_s = xs[s] @ K[s]; v_s = xs[s] @ V[s]   (per segment)
    scores[s] = dot(q_s, k_s)
    w = scores / ||scores||_2
    y[b] = sum_s w[s] * v_s            -> [E]

Sharding: data-parallel over B across 8 cores (512 rows each), Q/K/V
replicated. Host pre-transposes x to [S, D, B_loc] (contraction dim on
partitions) and casts inputs to bf16; accumulation is fp32 on device.

Self-contained: hardcodes all shapes; imports concourse from the system
install.
"""

import sys

import numpy as np
import ml_dtypes

for _p in ("/opt/trn_rl_repo",):
    if _p not in sys.path:
        sys.path.append(_p)

B, S, D, E = 4096, 32, 512, 512
NCORES = 8
BLOC = B // NCORES  # rows per core
P = 128             # partitions
DC = D // P         # contraction chunks per segment
BT = BLOC // P      # output row tiles per core

_BF16 = ml_dtypes.bfloat16

_nc_cache = None


def _build_bass():
    import concourse.bass as bass
    import concourse.mybir as mybir
    import concourse.tile as tile
    from concourse import bacc
    from concourse.bass import ts
    from contextlib import ExitStack

    fp32 = mybir.dt.float32
    bf16 = mybir.dt.bfloat16
    mult = mybir.AluOpType.mult
    add = mybir.AluOpType.add

    # Bacc (not raw Bass): its compile() pass splits multi-waits into
    # EventSemaphore insts (TRN2 allows 1 wait/inst) and lowers ISA ops.
    nc = bacc.Bacc("TRN2", debug=False)

    xt = nc.dram_tensor("xt", [S, D, BLOC], bf16, kind="ExternalInput")
    qd = nc.dram_tensor("qd", [S, D, E], bf16, kind="ExternalInput")
    kd = nc.dram_tensor("kd", [S, D, E], bf16, kind="ExternalInput")
    vd = nc.dram_tensor("vd", [S, D, E], bf16, kind="ExternalInput")
    yd = nc.dram_tensor("y", [BLOC, E], fp32, kind="ExternalOutput")

    yr = yd.rearrange("(t p) e -> t p e", p=P)

    with ExitStack() as ctx:
        tc = ctx.enter_context(tile.TileContext(nc))
        singles = ctx.enter_context(tc.tile_pool(name="singles", bufs=1))
        wpool = ctx.enter_context(tc.tile_pool(name="wmat", bufs=4))
        cpool = ctx.enter_context(tc.tile_pool(name="copies", bufs=3))
        spool = ctx.enter_context(tc.tile_pool(name="scratch", bufs=3))
        psum = ctx.enter_context(tc.tile_pool(name="psum", bufs=2, space="PSUM"))
        psum3 = ctx.enter_context(tc.tile_pool(name="psum3", bufs=4, space="PSUM"))

        # Residents: x^T for all segments (bf16, 128KB/partition), scores,
        # weights, output accumulator.
        xts = singles.tile([P, S, DC, BLOC], bf16)
        scores = singles.tile([P, BT, S], fp32)
        wts = singles.tile([P, BT, S], fp32)
        y_sb = singles.tile([P, BT, E], fp32)

        nc.vector.memset(y_sb, 0.0)

        # ---- pass 1: q/k projections + scores -------------------------
        for s in range(S):
            q_sb = wpool.tile([P, DC, E], bf16, tag="qk")
            k_sb = wpool.tile([P, DC, E], bf16, tag="qk")
            if s == 0:
                # chunk the very first loads so the first matmul can start
                # after ~256KB instead of 1.5MB
                for c in range(DC):
                    nc.sync.dma_start(out=xts[:, s, c], in_=xt[s, c * P : (c + 1) * P])
                    nc.sync.dma_start(out=q_sb[:, c], in_=qd[s, c * P : (c + 1) * P])
                    nc.sync.dma_start(out=k_sb[:, c], in_=kd[s, c * P : (c + 1) * P])
            else:
                nc.sync.dma_start(
                    out=xts[:, s], in_=xt[s].rearrange("(c p) b -> p c b", p=P)
                )
                nc.sync.dma_start(
                    out=q_sb, in_=qd[s].rearrange("(c p) e -> p c e", p=P)
                )
                nc.sync.dma_start(
                    out=k_sb, in_=kd[s].rearrange("(c p) e -> p c e", p=P)
                )
            for bt in range(BT):
                q_ps = psum.tile([P, E], fp32, tag="qps", bufs=3)
                k_ps = psum.tile([P, E], fp32, tag="kps", bufs=2)
                for c in range(DC):
                    nc.tensor.matmul(
                        q_ps,
                        xts[:, s, c, ts(bt, P)],
                        q_sb[:, c],
                        start=(c == 0),
                        stop=(c == DC - 1),
                    )
                for c in range(DC):
                    nc.tensor.matmul(
                        k_ps,
                        xts[:, s, c, ts(bt, P)],
                        k_sb[:, c],
                        start=(c == 0),
                        stop=(c == DC - 1),
                    )
                # scores[:, bt, s] = sum_e q*k ; one operand must live in
                # SBUF (DVE has a single PSUM read port), so stage q there
                # via the otherwise-idle Scalar engine.
                q_cp = cpool.tile([P, E], fp32, tag="qcp")
                nc.scalar.copy(q_cp, q_ps)
                junk = spool.tile([P, E], fp32, tag="junk")
                nc.vector.scalar_tensor_tensor(
                    out=junk,
                    in0=k_ps,
                    scalar=1.0,
                    in1=q_cp,
                    op0=mult,
                    op1=mult,
                    accum_out=scores[:, bt, s : s + 1],
                )

        # ---- normalize scores -> weights ------------------------------
        for bt in range(BT):
            sq = spool.tile([P, S], fp32, tag="normtmp")
            nsq = spool.tile([P, 1], fp32, tag="nsq")
            nc.vector.scalar_tensor_tensor(
                out=sq,
                in0=scores[:, bt],
                scalar=1.0,
                in1=scores[:, bt],
                op0=mult,
                op1=mult,
                accum_out=nsq,
            )
            nrm = spool.tile([P, 1], fp32, tag="nrm")
            nc.scalar.sqrt(nrm, nsq)
            rcp = spool.tile([P, 1], fp32, tag="rcp")
            nc.vector.reciprocal(rcp, nrm)
            nc.vector.tensor_scalar_mul(wts[:, bt], scores[:, bt], rcp)

        # ---- pass 2: v projection + weighted accumulation -------------
        for s in range(S):
            v_sb = wpool.tile([P, DC, E], bf16, tag="qk")
            nc.sync.dma_start(out=v_sb, in_=vd[s].rearrange("(c p) e -> p c e", p=P))
            for bt in range(BT):
                v_ps = psum3.tile([P, E], fp32, tag="vps", bufs=3)
                for c in range(DC):
                    nc.tensor.matmul(
                        v_ps,
                        xts[:, s, c, ts(bt, P)],
                        v_sb[:, c],
                        start=(c == 0),
                        stop=(c == DC - 1),
                    )
                # y[:, bt] += w[:, bt, s] * v
                nc.vector.scalar_tensor_tensor(
                    out=y_sb[:, bt],
                    in0=v_ps,
                    scalar=wts[:, bt, s : s + 1],
                    in1=y_sb[:, bt],
                    op0=mult,
                    op1=add,
                )

        # ---- store ----------------------------------------------------
        for bt in range(BT):
            nc.sync.dma_start(out=yr[bt], in_=y_sb[:, bt])

    # Run Bacc's compile passes (wait-splitting, ISA lowering, reg alloc).
    nc.finalize()
    return nc


def _get_nc():
    global _nc_cache
    if _nc_cache is None:
        _nc_cache = _build_bass()
    return _nc_cache


def _prep_in_maps(x, Q, K, V):
    x = np.asarray(x, dtype=np.float32)
    qb = np.ascontiguousarray(np.asarray(Q, dtype=np.float32)).astype(_BF16)
    kb = np.ascontiguousarray(np.asarray(K, dtype=np.float32)).astype(_BF16)
    vb = np.ascontiguousarray(np.asarray(V, dtype=np.float32)).astype(_BF16)
    in_maps = []
    for c in range(NCORES):
        xc = x[c * BLOC : (c + 1) * BLOC].reshape(BLOC, S, D)
        xtc = np.ascontiguousarray(xc.transpose(1, 2, 0)).astype(_BF16)  # [S, D, BLOC]
        in_maps.append({"xt": xtc, "qd": qb, "kd": kb, "vd": vb})
    return in_maps


def _run(in_maps, trace=False):
    from concourse.bass_utils import run_bass_kernel_spmd

    nc = _get_nc()
    res = run_bass_kernel_spmd(nc, in_maps, core_ids=list(range(NCORES)), trace=trace)
    y = np.concatenate([r["y"] for r in res.results], axis=0)
    return y, res


def kernel(x=None, Q=None, K=None, V=None, **_ignored):
    in_maps = _prep_in_maps(x, Q, K, V)
    y, _ = _run(in_maps, trace=False)
    return y


def kernel_traced(x, Q, K, V):
    in_maps = _prep_in_maps(x, Q, K, V)
    return _run(in_maps, trace=True)



# revision 6
# speedup vs baseline: 1.4335x; 1.4335x over previous
"""Trainium2 Bass kernel for the segmented-attention block.

Reference computation (per batch row b of x [B, S*D]):
    xs = x[b].reshape(S, D)
    q_s = xs[s] @ Q[s]; k_s = xs[s] @ K[s]; v_s = xs[s] @ V[s]   (per segment)
    scores[s] = dot(q_s, k_s)
    w = scores / ||scores||_2
    y[b] = sum_s w[s] * v_s            -> [E]

Math restructure: scores[s] = xs[s] @ (Q[s] K[s]^T) @ xs[s]^T. Only the
symmetric part A = (M + M^T)/2 of M = Q K^T contributes to a quadratic
form, and x^T A x = x^T Ã x exactly for Ã = 2*tril(A,-1) + diag(A),
whose upper triangle is zero. The host precomputes Ã once; the device
computes
    U = X_s @ Ã_s            (PE; upper-tri 128x128 blocks are zero and
                              skipped: 10 of 16 block-matmuls per tile)
    score = rowsum(U * X_s)  (DVE, needs a row-major copy of x)
vs the q/k/v formulation this cuts score-path matmul work by ~69% and
weight traffic by ~69%.

Sharding: data-parallel over B across 8 cores (512 rows each), M/V
replicated. Host supplies x twice: transposed [S, D, B_loc] for the PE
(contraction on partitions) and row-major [B_loc, S, D] for the DVE
rowsum. All inputs bf16; accumulation fp32 on device.

Self-contained: hardcodes all shapes; imports concourse from the system
install.
"""

import sys

import numpy as np
import ml_dtypes

for _p in ("/opt/trn_rl_repo",):
    if _p not in sys.path:
        sys.path.append(_p)

B, S, D, E = 4096, 32, 512, 512
NCORES = 8
BLOC = B // NCORES  # rows per core
P = 128             # partitions
DC = D // P         # contraction chunks per segment
BT = BLOC // P      # output row tiles per core

_BF16 = ml_dtypes.bfloat16

_nc_cache = None


def _build_bass():
    import concourse.bass as bass
    import concourse.mybir as mybir
    import concourse.tile as tile
    from concourse import bacc
    from concourse.bass import ts
    from contextlib import ExitStack

    fp32 = mybir.dt.float32
    bf16 = mybir.dt.bfloat16
    mult = mybir.AluOpType.mult
    add = mybir.AluOpType.add

    nc = bacc.Bacc("TRN2", debug=False)

    NBLK = DC * (DC + 1) // 2  # lower-tri 128x128 blocks of Ã per segment

    xt = nc.dram_tensor("xt", [S, D, BLOC], bf16, kind="ExternalInput")
    xr = nc.dram_tensor("xr", [BLOC, S, D], bf16, kind="ExternalInput")
    md = nc.dram_tensor("md", [S, NBLK, P, P], bf16, kind="ExternalInput")
    vd = nc.dram_tensor("vd", [S, D, E], bf16, kind="ExternalInput")
    yd = nc.dram_tensor("y", [BLOC, E], fp32, kind="ExternalOutput")

    yr = yd.rearrange("(t p) e -> t p e", p=P)

    with ExitStack() as ctx:
        tc = ctx.enter_context(tile.TileContext(nc))
        singles = ctx.enter_context(tc.tile_pool(name="singles", bufs=1))
        wpool = ctx.enter_context(tc.tile_pool(name="wmat", bufs=3))
        rpool = ctx.enter_context(tc.tile_pool(name="xrow", bufs=6))
        spool = ctx.enter_context(tc.tile_pool(name="scratch", bufs=2))
        psum = ctx.enter_context(tc.tile_pool(name="psum", bufs=4, space="PSUM"))
        psum2 = ctx.enter_context(tc.tile_pool(name="psum2", bufs=4, space="PSUM"))

        # Residents: x^T for all segments (bf16, 128KB/partition), scores,
        # weights, output accumulator.
        xts = singles.tile([P, S, DC, BLOC], bf16)
        scores = singles.tile([P, BT, S], fp32)
        wts = singles.tile([P, BT, S], fp32)
        y_sb = singles.tile([P, BT, E], fp32)

        nc.vector.memset(y_sb, 0.0)

        # ---- pass 1: U = X Ã, scores = rowsum(U * X) -------------------
        for s in range(S):
            m_sb = wpool.tile([P, NBLK, P], bf16, tag="w")
            if s == 0:
                # chunk the very first loads so the first matmul can start
                # after ~256KB instead of 1.3MB
                for c in range(DC):
                    nc.sync.dma_start(out=xts[:, s, c], in_=xt[s, c * P : (c + 1) * P])
                for b0 in range(0, NBLK, 3):
                    b1 = min(b0 + 3, NBLK)
                    nc.sync.dma_start(
                        out=m_sb[:, b0:b1],
                        in_=md[s, b0:b1].rearrange("t p e -> p t e"),
                    )
            else:
                nc.sync.dma_start(
                    out=xts[:, s], in_=xt[s].rearrange("(c p) b -> p c b", p=P)
                )
                nc.sync.dma_start(
                    out=m_sb, in_=md[s].rearrange("t p e -> p t e")
                )
            for bt in range(BT):
                rt = rpool.tile([P, D], bf16, tag="rt")
                nc.sync.dma_start(out=rt, in_=xr[ts(bt, P), s])
                t_ps = psum.tile([P, D], fp32, tag="tps")
                bidx = 0
                for j in range(DC):
                    nj = DC - j
                    for ii, i in enumerate(range(j, DC)):
                        nc.tensor.matmul(
                            t_ps[:, j * P : (j + 1) * P],
                            xts[:, s, i, ts(bt, P)],
                            m_sb[:, bidx],
                            start=(ii == 0),
                            stop=(ii == nj - 1),
                        )
                        bidx += 1
                junk = spool.tile([P, D], fp32, tag="junk")
                nc.vector.scalar_tensor_tensor(
                    out=junk,
                    in0=t_ps,
                    scalar=1.0,
                    in1=rt,
                    op0=mult,
                    op1=mult,
                    accum_out=scores[:, bt, s : s + 1],
                )

        # ---- normalize scores -> weights ------------------------------
        for bt in range(BT):
            sq = spool.tile([P, S], fp32, tag="normtmp")
            nsq = spool.tile([P, 1], fp32, tag="nsq")
            nc.vector.scalar_tensor_tensor(
                out=sq,
                in0=scores[:, bt],
                scalar=1.0,
                in1=scores[:, bt],
                op0=mult,
                op1=mult,
                accum_out=nsq,
            )
            nrm = spool.tile([P, 1], fp32, tag="nrm")
            nc.scalar.sqrt(nrm, nsq)
            rcp = spool.tile([P, 1], fp32, tag="rcp")
            nc.vector.reciprocal(rcp, nrm)
            nc.vector.tensor_scalar_mul(wts[:, bt], scores[:, bt], rcp)

        # ---- pass 2: v projection + weighted accumulation -------------
        for s in range(S):
            v_sb = wpool.tile([P, DC, E], bf16, tag="w")
            nc.sync.dma_start(out=v_sb, in_=vd[s].rearrange("(c p) e -> p c e", p=P))
            for bt in range(BT):
                v_ps = psum2.tile([P, E], fp32, tag="vps")
                for c in range(DC):
                    nc.tensor.matmul(
                        v_ps,
                        xts[:, s, c, ts(bt, P)],
                        v_sb[:, c],
                        start=(c == 0),
                        stop=(c == DC - 1),
                    )
                # y[:, bt] += w[:, bt, s] * v
                nc.vector.scalar_tensor_tensor(
                    out=y_sb[:, bt],
                    in0=v_ps,
                    scalar=wts[:, bt, s : s + 1],
                    in1=y_sb[:, bt],
                    op0=mult,
                    op1=add,
                )

        # ---- store ----------------------------------------------------
        for bt in range(BT):
            nc.sync.dma_start(out=yr[bt], in_=y_sb[:, bt])

    nc.finalize()
    return nc


def _get_nc():
    global _nc_cache
    if _nc_cache is None:
        _nc_cache = _build_bass()
    return _nc_cache


def _prep_in_maps(x, Q, K, V):
    x = np.asarray(x, dtype=np.float32)
    Qf = np.ascontiguousarray(np.asarray(Q, dtype=np.float32))
    Kf = np.ascontiguousarray(np.asarray(K, dtype=np.float32))
    M = np.matmul(Qf, Kf.transpose(0, 2, 1))  # [S, D, D]
    # x^T M x == x^T Ã x for Ã = tril(M + M^T, -1) + diag(M): fold the
    # upper triangle onto the lower so upper-tri blocks vanish.
    At = np.tril(M + M.transpose(0, 2, 1), -1)
    ii = np.arange(D)
    At[:, ii, ii] = M[:, ii, ii]
    # pack lower-tri 128x128 blocks, j-major then i=j..DC-1
    DCn = D // P
    blocks = []
    for j in range(DCn):
        for i in range(j, DCn):
            blocks.append(At[:, i * P : (i + 1) * P, j * P : (j + 1) * P])
    mb = np.ascontiguousarray(
        np.stack(blocks, axis=1)  # [S, NBLK, P, P]
    ).astype(_BF16)
    vb = np.ascontiguousarray(np.asarray(V, dtype=np.float32)).astype(_BF16)
    in_maps = []
    for c in range(NCORES):
        xc = x[c * BLOC : (c + 1) * BLOC].reshape(BLOC, S, D)
        xtc = np.ascontiguousarray(xc.transpose(1, 2, 0)).astype(_BF16)  # [S, D, BLOC]
        xrc = xc.astype(_BF16)  # [BLOC, S, D]
        in_maps.append({"xt": xtc, "xr": xrc, "md": mb, "vd": vb})
    return in_maps


def _run(in_maps, trace=False):
    from concourse.bass_utils import run_bass_kernel_spmd

    nc = _get_nc()
    res = run_bass_kernel_spmd(nc, in_maps, core_ids=list(range(NCORES)), trace=trace)
    y = np.concatenate([r["y"] for r in res.results], axis=0)
    return y, res


def kernel(x=None, Q=None, K=None, V=None, **_ignored):
    in_maps = _prep_in_maps(x, Q, K, V)
    y, _ = _run(in_maps, trace=False)
    return y


def kernel_traced(x, Q, K, V):
    in_maps = _prep_in_maps(x, Q, K, V)
    return _run(in_maps, trace=True)
